# revision 14
# baseline (speedup 1.0000x reference)
"""Trainium2 Bass kernel for ColorFlowLayer GNN message passing.

Design (8 NeuronCores, SPMD; wall-clock over the axon tunnel is the
bottleneck at ~42 MB/s, so the kernel minimizes host<->device bytes and
the number of transferred arrays):
  - Ship per-core ONLY two packed blobs: fp16 (h shard, window one-hot
    keys, folded weights) and uint16 (edge indices; rel packed into the
    high 3 bits of dst).  ~2.5 MB/core in, 1.6 MB/core out.
  - On device: AllGather h shards (DRAM bounce -> Shared), build fp16
    feature-major tables in SBUF:
      A.T [128, 50176]  = (h @ eW1[0:128]).T    (+ role/color combos)
      B'.T [128, 6272]  = (h_loc @ eW1[128:256]).T (+ role/color combos)
      znh.T [128, 6272] = (h_loc @ nW1[0:128]).T (+ role/color + nb1)
    Role/color/rel embedding contributions enter via 18/8-column combo
    tables gathered by host-precomputed uint16 codes.
  - Edge phase: edges sorted by dst, padded into 128-edge tiles that
    never span a 128-node window; per 1024-edge block, gpsimd
    indirect_copy gathers per-edge columns of A.T (7 zero-padded
    chunks <= 8066 cols, summed), B'.T, Rtab.T; z = sum -> silu ->
    y1 @ eW2 -> silu -> one-hot segment-sum matmul into PSUM.
  - Node phase per 128-node window: zn = nW1_agg.T @ agg + znh ->
    silu -> @ nW2 -> + h -> LayerNorm (f32) -> fp16 out.
"""

import numpy as np

H = 128
P = 128
NCORES = 8
NS = 6272            # padded nodes per core = 49 windows * 128
NWL = NS // P        # 49 local windows
NWG = NWL * NCORES   # 392 global windows
NG = NS * NCORES     # 50176 padded global nodes
BLK = 8              # edge tiles per block (1024 edges)
CH = 8064            # A-table chunk data columns (63 windows)
NCHUNK = 7           # 6*8064 + 1792 = 50176
LCH = NG - 6 * CH    # 1792, last chunk data cols
LN_EPS = 1e-5

_CACHE = {}
_LAST_EXEC_NS = None


def _offsets(NT):
    """Row offsets (rows of 128 elements) into the two packed blobs."""
    oF = {}
    r = 0
    for name, rows in [("hs", NWL), ("W1_hs", H),
                       ("W1_hd", H), ("eW2", H), ("nW1_h", H),
                       ("nW1_agg", H), ("nW2", H), ("RtabT", 8),
                       ("ACT", 18), ("BCT", 18), ("NTT", 18),
                       ("eb2row", 1), ("nb2row", 1), ("lng", 1), ("lnb", 1)]:
        oF[name] = (r, rows)
        r += rows
    oU = {}
    r = 0
    for name, rows in [("srcv", NT), ("dstrv", NT), ("rc_all", NWG),
                       ("rc_loc", NWL)]:
        oU[name] = (r, rows)
        r += rows
    return oF, oU


def _prep_host(h, edge_index, edge_relation, node_color_rep, node_role,
               rel_emb, role_emb, color_emb,
               eW1, eb1, eW2, eb2, nW1, nb1, nW2, nb2, ln_g, ln_b):
    f32, f16, u16 = np.float32, np.float16, np.uint16
    h = np.asarray(h, f32)
    src = np.asarray(edge_index[0], np.int64)
    dst = np.asarray(edge_index[1], np.int64)
    rel = np.asarray(edge_relation, np.int64)
    role = np.asarray(node_role, np.int64)
    col = np.asarray(node_color_rep, np.int64)
    N = h.shape[0]
    E = src.shape[0]

    # ---- folded weights (tiny) ----
    eW1 = np.asarray(eW1, f32)
    nW1 = np.asarray(nW1, f32)
    Rtab = np.asarray(rel_emb, f32) @ eW1[256:272] + np.asarray(eb1, f32)
    RA = np.asarray(role_emb, f32) @ eW1[272:280]
    RB = np.asarray(role_emb, f32) @ eW1[280:288]
    CA = np.asarray(color_emb, f32) @ eW1[288:296]
    CB = np.asarray(color_emb, f32) @ eW1[296:304]
    NR = np.asarray(role_emb, f32) @ nW1[256:264] + np.asarray(nb1, f32)
    NC = np.asarray(color_emb, f32) @ nW1[264:272]
    AC18 = (RA[:, None, :] + CA[None, :, :]).reshape(18, H)
    BC18 = (RB[:, None, :] + CB[None, :, :]).reshape(18, H)
    NT18 = (NR[:, None, :] + NC[None, :, :]).reshape(18, H)

    eb2 = np.asarray(eb2, f32)
    nb2 = np.asarray(nb2, f32)
    has_eb2 = bool(np.any(eb2 != 0))
    has_nb2 = bool(np.any(nb2 != 0))
    ln_g = np.asarray(ln_g, f32)
    ln_b = np.asarray(ln_b, f32)
    ln_id = bool(np.all(ln_g == 1) and np.all(ln_b == 0))

    # ---- edge sharding: sort globally by dst (core = dst // NS) ----
    o = np.argsort(dst, kind="stable")
    src_s, dst_s, rel_s = src[o], dst[o], rel[o]
    core_s = dst_s // NS
    gw = dst_s // P                       # global window id 0..391
    wcnt = np.bincount(gw, minlength=NWG)
    cnts = wcnt.reshape(NCORES, NWL)      # [core, local window]
    T = np.maximum(1, np.ceil(cnts.max(axis=0) / P).astype(np.int64))
    NT = int(T.sum())
    NT += (-NT) % BLK
    T[NWL - 1] += NT - int(T.sum())
    offs = np.concatenate([[0], np.cumsum(T)]).astype(np.int64)
    NB = NT // BLK

    starts = np.concatenate([[0], np.cumsum(wcnt)]).astype(np.int64)
    rank = np.arange(E) - starts[gw]
    slot = offs[gw % NWL] * P + rank      # slot within the core's edge space

    srcv = np.zeros((NCORES, NT * P), u16)
    dstrv = np.full((NCORES, NT * P), 8191, u16)   # pad marker: dst=8191
    srcv[core_s, slot] = src_s.astype(u16)
    dstrv[core_s, slot] = (rel_s * 8192 + dst_s - core_s * NS).astype(u16)

    def tilecm(a):  # [NT*P] slot-major -> [P, NT] (slot t*128+p at [p, t])
        return np.ascontiguousarray(a.reshape(NT, P).T)

    # ---- per-node role-color codes ----
    rc = np.zeros(NG, u16)
    rc[:N] = (role * 3 + col).astype(u16)
    rc_all = np.ascontiguousarray(rc.reshape(NWG, P).T)  # [P, 392]

    # per-row int8 quantization of h
    rowmax = np.maximum(np.abs(h).max(axis=1), 1e-4)
    hscale = (rowmax / 127.0).astype(f16)
    h8 = np.round(h / (rowmax / 127.0)[:, None]).astype(np.int8)
    h8_pad = np.zeros((NG, H), np.int8)
    h8_pad[:N] = h8
    hs_pad = np.full(NG, 1e-4, f16)
    hs_pad[:N] = hscale

    oF, oU = _offsets(NT)
    rowsF = sum(n for _, n in oF.values())
    rowsU = sum(n for _, n in oU.values())

    fixedF = {
        "W1_hs": eW1[0:128].astype(f16),
        "W1_hd": eW1[128:256].astype(f16),
        "eW2": np.asarray(eW2, f32).astype(f16),
        "nW1_h": nW1[0:128].astype(f16),
        "nW1_agg": nW1[128:256].astype(f16),
        "nW2": np.asarray(nW2, f32).astype(f16),
        "RtabT": np.ascontiguousarray(Rtab.T).astype(f16),
        "ACT": np.ascontiguousarray(AC18.T).astype(f16),
        "BCT": np.ascontiguousarray(BC18.T).astype(f16),
        "NTT": np.ascontiguousarray(NT18.T).astype(f16),
        "eb2row": eb2.reshape(1, H).astype(f16),
        "nb2row": nb2.reshape(1, H).astype(f16),
        "lng": ln_g.reshape(1, H).astype(f16),
        "lnb": ln_b.reshape(1, H).astype(f16),
    }

    ins_per_core = []
    for c in range(NCORES):
        blobF = np.empty((rowsF, P), f16)
        blobU = np.empty((rowsU, P), u16)

        def putF(name, arr):
            r0, nr = oF[name]
            blobF[r0:r0 + nr] = np.asarray(arr).reshape(nr, P)

        def putU(name, arr):
            r0, nr = oU[name]
            blobU[r0:r0 + nr] = np.asarray(arr).reshape(nr, P)

        putF("hs", hs_pad[c * NS:(c + 1) * NS])
        for kk, vv in fixedF.items():
            putF(kk, vv)
        putU("srcv", tilecm(srcv[c]))
        putU("dstrv", tilecm(dstrv[c]))
        putU("rc_all", rc_all)
        putU("rc_loc", rc_all[:, c * NWL:(c + 1) * NWL])
        ins_per_core.append(dict(
            blobF=blobF, blobU=blobU,
            blobQ=np.ascontiguousarray(h8_pad[c * NS:(c + 1) * NS])))
    meta = dict(NT=NT, T=tuple(int(t) for t in T),
                has_eb2=has_eb2, has_nb2=has_nb2, ln_id=ln_id)
    return ins_per_core, meta, N


def _build_nc(meta):
    import concourse.bass as bass
    import concourse.bacc as bacc
    import concourse.mybir as mybir
    import concourse.tile as tile
    from concourse.masks import make_identity
    from contextlib import ExitStack

    NT = meta["NT"]
    T = meta["T"]
    NB = NT // BLK
    AF = mybir.ActivationFunctionType
    ALU = mybir.AluOpType
    dt = mybir.dt
    nc = bacc.Bacc()

    oF, oU = _offsets(NT)
    rowsF = sum(n for _, n in oF.values())
    rowsU = sum(n for _, n in oU.values())
    blobF_d = nc.dram_tensor("blobF", [rowsF, P], dt.float16,
                             kind="ExternalInput")
    blobU_d = nc.dram_tensor("blobU", [rowsU, P], dt.uint16,
                             kind="ExternalInput")
    blobQ_d = nc.dram_tensor("blobQ", [NS, H], dt.int8,
                             kind="ExternalInput")

    def fsl(name):
        r0, nr = oF[name]
        return blobF_d[r0:r0 + nr, :]

    def usl(name):
        r0, nr = oU[name]
        return blobU_d[r0:r0 + nr, :]

    outq_d = nc.dram_tensor("outq", [NS, H], dt.int8, kind="ExternalOutput")
    outs_d = nc.dram_tensor("outs", [NS, 1], dt.float16,
                            kind="ExternalOutput")

    ag_in = nc.dram_tensor("ag_in", [NS, H], dt.int8)
    ag_ins = nc.dram_tensor("ag_ins", [NS, 1], dt.float16)
    h_all = nc.dram_tensor("h_all", [NG, H], dt.int8, addr_space="Shared")
    hs_all = nc.dram_tensor("hs_all", [NG, 1], dt.float16,
                            addr_space="Shared")

    ACH = 6 * (CH + 2) + (LCH + 2)   # flat A-table cols incl zero pads

    with tile.TileContext(nc) as tc, ExitStack() as ctx:
        cst = ctx.enter_context(tc.tile_pool(name="cst", bufs=1))
        big = ctx.enter_context(tc.tile_pool(name="big", bufs=1))

        # ---- start the collectives as early as possible ----
        nc.sync.dma_start(ag_in[:], blobQ_d[:])
        nc.sync.dma_start(ag_ins[:], fsl("hs"))
        tc.strict_bb_all_engine_barrier()
        nc.gpsimd.collective_compute(
            "AllGather", mybir.AluOpType.bypass,
            replica_groups=[list(range(NCORES))],
            ins=[ag_in[:]], outs=[h_all[:]])
        nc.gpsimd.collective_compute(
            "AllGather", mybir.AluOpType.bypass,
            replica_groups=[list(range(NCORES))],
            ins=[ag_ins[:]], outs=[hs_all[:]])

        # ---- constants ----
        ident = cst.tile([P, P], dt.float16)
        make_identity(nc, ident[:])
        W1_hs = cst.tile([H, H], dt.float16)
        W1_hd = cst.tile([H, H], dt.float16)
        eW2 = cst.tile([H, H], dt.float16)
        nW1_h = cst.tile([H, H], dt.float16)
        nW1_agg = cst.tile([H, H], dt.float16)
        nW2 = cst.tile([H, H], dt.float16)
        RtabT = cst.tile([P, 8], dt.float16)
        ACT = cst.tile([P, 18], dt.float16)
        BCT = cst.tile([P, 18], dt.float16)
        NTT = cst.tile([P, 18], dt.float16)
        eb2r = cst.tile([1, H], dt.float16)
        nb2r = cst.tile([1, H], dt.float16)
        lngr = cst.tile([1, H], dt.float16)
        lnbr = cst.tile([1, H], dt.float16)
        ones1h = cst.tile([1, P], dt.float16)
        for nm, dstt in [("W1_hs", W1_hs), ("W1_hd", W1_hd), ("eW2", eW2),
                         ("nW1_h", nW1_h), ("nW1_agg", nW1_agg),
                         ("nW2", nW2), ("RtabT", RtabT), ("ACT", ACT),
                         ("BCT", BCT), ("NTT", NTT), ("eb2row", eb2r),
                         ("nb2row", nb2r), ("lng", lngr), ("lnb", lnbr)]:
            nc.sync.dma_start(dstt[:], fsl(nm))
        nc.vector.memset(ones1h[:], 1.0)

        iota16 = cst.tile([P, BLK, P], dt.float16)

        # LN gamma/beta broadcast to [P, H] f32 via ones-matmul
        lng = cst.tile([P, H], dt.float32)
        lnb = cst.tile([P, H], dt.float32)
        with tc.tile_pool(name="lnp", bufs=2, space="PSUM") as lnp:
            pg = lnp.tile([P, H], dt.float32, tag="g")
            nc.tensor.matmul(out=pg[:], lhsT=ones1h[:], rhs=lngr[:],
                             start=True, stop=True)
            nc.vector.tensor_copy(out=lng[:], in_=pg[:])
            pb = lnp.tile([P, H], dt.float32, tag="b")
            nc.tensor.matmul(out=pb[:], lhsT=ones1h[:], rhs=lnbr[:],
                             start=True, stop=True)
            nc.vector.tensor_copy(out=lnb[:], in_=pb[:])

        # ---- persistent tables ----
        tblA = big.tile([P, ACH], dt.float16)
        nc.vector.memset(tblA[:], 0.0)
        tblB = big.tile([P, NS], dt.float16)
        znh = big.tile([P, NS], dt.float16)
        h_raw = big.tile([P, NWL, H], dt.float16)
        dstwv_w = big.tile([P, NB, BLK], dt.float16)
        srcW = big.tile([P, NB, 64], dt.uint16)
        dstW = big.tile([P, NB, 64], dt.uint16)
        relW = big.tile([P, NB, 64], dt.uint16)
        WrcA = big.tile([P, NWG, 8], dt.uint16)
        WrcL = big.tile([P, NWL, 8], dt.uint16)
        with tc.tile_pool(name="stg", bufs=1) as stg:
            iota_g = stg.tile([P, BLK, P], dt.float16)
            nc.gpsimd.iota(iota_g[:], pattern=[[0, BLK], [1, P]], base=0,
                           channel_multiplier=0,
                           allow_small_or_imprecise_dtypes=True)
            # DVE-owned copy: the 3D-broadcast is_equal only has room for
            # one sync wait, so both inputs must come from DVE producers.
            nc.vector.tensor_copy(out=iota16[:], in_=iota_g[:])
            srcv = stg.tile([P, NB, BLK], dt.uint16)
            dstvt = stg.tile([P, NB, BLK], dt.uint16)
            rc_all = stg.tile([P, NWG], dt.uint16)
            rc_loc = stg.tile([P, NWL], dt.uint16)
            nc.sync.dma_start(srcv[:], usl("srcv"))
            nc.sync.dma_start(dstvt[:], usl("dstrv"))
            nc.sync.dma_start(rc_all[:], usl("rc_all"))
            nc.sync.dma_start(rc_loc[:], usl("rc_loc"))
            # W[q, blk, tl*8+cp] = v[cp*16+q, blk, tl]
            for tl in range(BLK):
                for cp in range(8):
                    c = tl * 8 + cp
                    sl = slice(cp * 16, cp * 16 + 16)
                    nc.sync.dma_start(srcW[0:16, :, c:c + 1],
                                      srcv[sl, :, tl:tl + 1])
                    nc.sync.dma_start(dstW[0:16, :, c:c + 1],
                                      dstvt[sl, :, tl:tl + 1])
            for g in range(1, 8):
                gs = slice(16 * g, 16 * (g + 1))
                nc.sync.dma_start(srcW[gs, :, :], srcW[0:16, :, :])
                nc.sync.dma_start(dstW[gs, :, :], dstW[0:16, :, :])
            # unpack rel (high 3 bits) out of dstW; clamp pads to 6271
            nc.vector.tensor_scalar(out=relW[:], in0=dstW[:], scalar1=13,
                                    scalar2=0,
                                    op0=ALU.logical_shift_right,
                                    op1=ALU.bypass)
            nc.vector.tensor_scalar(out=dstW[:], in0=dstW[:], scalar1=8191,
                                    scalar2=0,
                                    op0=ALU.bitwise_and, op1=ALU.bypass)
            nc.vector.tensor_scalar(out=dstW[:], in0=dstW[:],
                                    scalar1=NS - 1, scalar2=0,
                                    op0=ALU.min, op1=ALU.bypass)
            # derive window one-hot keys from compact dstrv:
            # m = dst&8191; c = (m < NS); dstwv = (m&127 + 1)*c - 1
            dm = stg.tile([P, NB, BLK], dt.uint16)
            nc.vector.tensor_scalar(out=dm[:], in0=dstvt[:], scalar1=8191,
                                    scalar2=0, op0=ALU.bitwise_and,
                                    op1=ALU.bypass)
            dc = stg.tile([P, NB, BLK], dt.uint16)
            nc.vector.tensor_scalar(out=dc[:], in0=dm[:], scalar1=NS,
                                    scalar2=0, op0=ALU.is_lt,
                                    op1=ALU.bypass)
            dw1 = stg.tile([P, NB, BLK], dt.uint16)
            nc.vector.tensor_scalar(out=dw1[:], in0=dm[:], scalar1=127,
                                    scalar2=0, op0=ALU.bitwise_and,
                                    op1=ALU.bypass)
            nc.vector.tensor_scalar(out=dw1[:], in0=dw1[:], scalar1=1,
                                    scalar2=0, op0=ALU.add,
                                    op1=ALU.bypass)
            dwf = stg.tile([P, NB, BLK], dt.float16)
            nc.vector.tensor_tensor(out=dwf[:], in0=dw1[:], in1=dc[:],
                                    op=ALU.mult)
            nc.vector.tensor_scalar(out=dstwv_w[:], in0=dwf[:],
                                    scalar1=1.0, scalar2=0.0,
                                    op0=ALU.subtract, op1=ALU.bypass)
            # window-granular wraps for rc: Wrc[q, w, cp] = rc[cp*16+q, w]
            for cp in range(8):
                sl = slice(cp * 16, cp * 16 + 16)
                nc.sync.dma_start(WrcA[0:16, :, cp:cp + 1],
                                  rc_all[sl, :].unsqueeze(2))
                nc.sync.dma_start(WrcL[0:16, :, cp:cp + 1],
                                  rc_loc[sl, :].unsqueeze(2))
            for g in range(1, 8):
                gs = slice(16 * g, 16 * (g + 1))
                nc.sync.dma_start(WrcA[gs, :, :], WrcA[0:16, :, :])
                nc.sync.dma_start(WrcL[gs, :, :], WrcL[0:16, :, :])

        # ---- local phase: h_raw, tblB, znh from int8 h ----
        hsr0 = oF["hs"][0]
        with tc.tile_pool(name="tpp", bufs=2, space="PSUM") as tpp, \
             tc.tile_pool(name="tbp", bufs=2, space="PSUM") as tbp, \
             tc.tile_pool(name="lsb", bufs=3) as lsb:
            for w in range(NWL):
                q8w = lsb.tile([P, H], dt.int8, tag="q8w")
                nc.sync.dma_start(q8w[:],
                                  blobQ_d[w * P:(w + 1) * P, :])
                hsw = lsb.tile([P, 1], dt.float16, tag="hsw")
                nc.sync.dma_start(
                    hsw[:], blobF_d[hsr0 + w:hsr0 + w + 1, :])
                hsw32 = lsb.tile([P, 1], dt.float32, tag="hsw32")
                nc.vector.tensor_copy(out=hsw32[:], in_=hsw[:])
                hqf = lsb.tile([P, H], dt.float16, tag="hqf")
                nc.vector.tensor_copy(out=hqf[:], in_=q8w[:])
                nc.vector.tensor_scalar_mul(h_raw[:, w, :], hqf[:],
                                            hsw32[:])
                pt = tpp.tile([P, P], dt.float16, tag="tr")
                nc.tensor.transpose(out=pt[:], in_=h_raw[:, w, :],
                                    identity=ident[:])
                hT = lsb.tile([P, P], dt.float16, tag="hT")
                nc.vector.tensor_copy(out=hT[:], in_=pt[:])
                pb = tbp.tile([P, P], dt.float32, tag="pb")
                nc.tensor.matmul(out=pb[:], lhsT=W1_hd[:], rhs=hT[:],
                                 start=True, stop=True)
                gb = lsb.tile([P, P], dt.float16, tag="gb")
                nc.gpsimd.indirect_copy(out=gb[:], data=BCT[:],
                                        idxs=WrcL[:, w, :],
                                        i_know_ap_gather_is_preferred=True)
                nc.vector.tensor_add(out=tblB[:, w * P:(w + 1) * P],
                                     in0=pb[:], in1=gb[:])
                pz = tbp.tile([P, P], dt.float32, tag="pz")
                nc.tensor.matmul(out=pz[:], lhsT=nW1_h[:], rhs=hT[:],
                                 start=True, stop=True)
                gz = lsb.tile([P, P], dt.float16, tag="gz")
                nc.gpsimd.indirect_copy(out=gz[:], data=NTT[:],
                                        idxs=WrcL[:, w, :],
                                        i_know_ap_gather_is_preferred=True)
                nc.vector.tensor_add(out=znh[:, w * P:(w + 1) * P],
                                     in0=pz[:], in1=gz[:])

        tc.strict_bb_all_engine_barrier()

        # ---- global phase: tblA from h_all ----
        with tc.tile_pool(name="gpp", bufs=2, space="PSUM") as gpp, \
             tc.tile_pool(name="gap", bufs=2, space="PSUM") as gap, \
             tc.tile_pool(name="gsb", bufs=3) as gsb:
            for w in range(NWG):
                k = w // 63
                off = k * (CH + 2) + (w % 63) * P + 1
                q8g = gsb.tile([P, P], dt.int8, tag="q8g")
                nc.sync.dma_start(q8g[:], h_all[w * P:(w + 1) * P, :])
                hsg = gsb.tile([P, 1], dt.float16, tag="hsg")
                nc.sync.dma_start(hsg[:], hs_all[w * P:(w + 1) * P, :])
                hsg32 = gsb.tile([P, 1], dt.float32, tag="hsg32")
                nc.vector.tensor_copy(out=hsg32[:], in_=hsg[:])
                hqg = gsb.tile([P, P], dt.float16, tag="hqg")
                nc.vector.tensor_copy(out=hqg[:], in_=q8g[:])
                ht = gsb.tile([P, P], dt.float16, tag="ld")
                nc.vector.tensor_scalar_mul(ht[:], hqg[:], hsg32[:])
                pt = gpp.tile([P, P], dt.float16, tag="tr")
                nc.tensor.transpose(out=pt[:], in_=ht[:], identity=ident[:])
                hT = gsb.tile([P, P], dt.float16, tag="hT")
                nc.vector.tensor_copy(out=hT[:], in_=pt[:])
                pa = gap.tile([P, P], dt.float32, tag="pa")
                nc.tensor.matmul(out=pa[:], lhsT=W1_hs[:], rhs=hT[:],
                                 start=True, stop=True)
                ga = gsb.tile([P, P], dt.float16, tag="ga")
                nc.gpsimd.indirect_copy(out=ga[:], data=ACT[:],
                                        idxs=WrcA[:, w, :],
                                        i_know_ap_gather_is_preferred=True)
                nc.vector.tensor_add(out=tblA[:, off:off + P],
                                     in0=pa[:], in1=ga[:])

        # ---- edge + node phases ----
        w_first = {}
        w_last = {}
        t2w = []
        for w in range(NWL):
            for _ in range(T[w]):
                t2w.append(w)
        for t, w in enumerate(t2w):
            w_first.setdefault(w, t)
            w_last[w] = t

        with tc.tile_pool(name="wkp", bufs=3) as wkp, \
             tc.tile_pool(name="zp", bufs=2) as zp, \
             tc.tile_pool(name="gp", bufs=1) as gp, \
             tc.tile_pool(name="y1p", bufs=2) as y1p, \
             tc.tile_pool(name="ohp", bufs=2) as ohp, \
             tc.tile_pool(name="msp", bufs=2) as msp, \
             tc.tile_pool(name="nod", bufs=1) as nod, \
             tc.tile_pool(name="mps", bufs=2, space="PSUM") as mps, \
             tc.tile_pool(name="aps", bufs=1, space="PSUM") as aps, \
             tc.tile_pool(name="nps", bufs=2, space="PSUM") as nps:

            agg_ps = None
            for b in range(NB):
                t0 = b * BLK
                z = zp.tile([P, BLK * P], dt.float16, tag="z")
                gt = gp.tile([P, BLK * P], dt.float16, tag="gt")
                for k in range(NCHUNK):
                    wk = wkp.tile([P, 64], dt.uint16, tag="wk")
                    lim = (CH + 1) if k < 6 else (LCH + 1)
                    if k == 0:
                        nc.vector.tensor_scalar(
                            out=wk[:], in0=srcW[:, b, :], scalar1=1,
                            scalar2=lim, op0=ALU.add, op1=ALU.min)
                    else:
                        nc.vector.tensor_scalar(
                            out=wk[:], in0=srcW[:, b, :],
                            scalar1=k * CH - 1, scalar2=lim,
                            op0=ALU.subtract, op1=ALU.min)
                    koff = k * (CH + 2)
                    klen = (CH + 2) if k < 6 else (LCH + 2)
                    dslice = tblA[:, koff:koff + klen]
                    if k == 0:
                        nc.gpsimd.indirect_copy(
                            out=z[:], data=dslice, idxs=wk[:],
                            i_know_ap_gather_is_preferred=True)
                    else:
                        nc.gpsimd.indirect_copy(
                            out=gt[:], data=dslice, idxs=wk[:],
                            i_know_ap_gather_is_preferred=True)
                        nc.vector.tensor_add(out=z[:], in0=z[:], in1=gt[:])
                gb = gp.tile([P, BLK * P], dt.float16, tag="gb")
                nc.gpsimd.indirect_copy(
                    out=gb[:], data=tblB[:], idxs=dstW[:, b, :],
                    i_know_ap_gather_is_preferred=True)
                nc.vector.tensor_add(out=z[:], in0=z[:], in1=gb[:])
                gr = gp.tile([P, BLK * P], dt.float16, tag="gr")
                nc.gpsimd.indirect_copy(
                    out=gr[:], data=RtabT[:], idxs=relW[:, b, :],
                    i_know_ap_gather_is_preferred=True)
                nc.vector.tensor_add(out=z[:], in0=z[:], in1=gr[:])

                y1 = y1p.tile([P, BLK * P], dt.float16, tag="y1")
                nc.scalar.activation(y1[:], z[:], AF.Silu)

                oh = ohp.tile([P, BLK, P], dt.float16, tag="oh")
                nc.vector.tensor_tensor(
                    out=oh[:],
                    in0=dstwv_w[:, b, :].unsqueeze(2).to_broadcast(
                        [P, BLK, P]),
                    in1=iota16[:],
                    op=ALU.is_equal)

                for half in range(2):
                    mp = mps.tile([P, 4 * P], dt.float32, tag="m")
                    for s4 in range(4):
                        s = half * 4 + s4
                        nc.tensor.matmul(out=mp[:, s4 * P:(s4 + 1) * P],
                                         lhsT=y1[:, s * P:(s + 1) * P],
                                         rhs=eW2[:],
                                         start=True, stop=not meta["has_eb2"])
                        if meta["has_eb2"]:
                            nc.tensor.matmul(out=mp[:, s4 * P:(s4 + 1) * P],
                                             lhsT=ones1h[:], rhs=eb2r[:],
                                             start=False, stop=True)
                    ms = msp.tile([P, 4 * P], dt.float16, tag="ms")
                    nc.scalar.activation(ms[:], mp[:], AF.Silu)
                    for s4 in range(4):
                        s = half * 4 + s4
                        t = t0 + s
                        w = t2w[t]
                        if t == w_first[w]:
                            agg_ps = aps.tile([P, P], dt.float32, tag="agg")
                        nc.tensor.matmul(out=agg_ps[:],
                                         lhsT=ms[:, s4 * P:(s4 + 1) * P],
                                         rhs=oh[:, s, :],
                                         start=(t == w_first[w]),
                                         stop=(t == w_last[w]))
                        if t != w_last[w]:
                            continue
                        # ---------- node phase for window w ----------
                        aggT = nod.tile([P, P], dt.float16, tag="aggT")
                        nc.vector.tensor_copy(out=aggT[:], in_=agg_ps[:])
                        zn = nps.tile([P, P], dt.float32, tag="zn")
                        nc.tensor.matmul(out=zn[:], lhsT=nW1_agg[:],
                                         rhs=aggT[:], start=True, stop=True)
                        zs = nod.tile([P, P], dt.float16, tag="zs")
                        nc.vector.tensor_add(
                            out=zs[:], in0=znh[:, w * P:(w + 1) * P],
                            in1=zn[:])
                        y1n = nod.tile([P, P], dt.float16, tag="y1n")
                        nc.scalar.activation(y1n[:], zs[:], AF.Silu)
                        up = nps.tile([P, P], dt.float32, tag="up")
                        nc.tensor.matmul(out=up[:], lhsT=y1n[:], rhs=nW2[:],
                                         start=True,
                                         stop=not meta["has_nb2"])
                        if meta["has_nb2"]:
                            nc.tensor.matmul(out=up[:], lhsT=ones1h[:],
                                             rhs=nb2r[:], start=False,
                                             stop=True)
                        x = nod.tile([P, H], dt.float32, tag="x")
                        nc.vector.tensor_add(out=x[:], in0=up[:],
                                             in1=h_raw[:, w, :])
                        mu = nod.tile([P, 1], dt.float32, tag="mu")
                        nc.vector.reduce_sum(out=mu[:], in_=x[:],
                                             axis=mybir.AxisListType.X)
                        nc.vector.tensor_scalar_mul(mu[:], mu[:], -1.0 / H)
                        xc = nod.tile([P, H], dt.float32, tag="xc")
                        nc.vector.tensor_scalar_add(xc[:], x[:], mu[:])
                        sq = nod.tile([P, H], dt.float32, tag="sq")
                        nc.vector.tensor_mul(out=sq[:], in0=xc[:], in1=xc[:])
                        var = nod.tile([P, 1], dt.float32, tag="var")
                        nc.vector.reduce_sum(out=var[:], in_=sq[:],
                                             axis=mybir.AxisListType.X)
                        nc.vector.tensor_scalar(
                            out=var[:], in0=var[:],
                            scalar1=1.0 / H, scalar2=LN_EPS,
                            op0=ALU.mult, op1=ALU.add)
                        std = nod.tile([P, 1], dt.float32, tag="std")
                        nc.scalar.activation(std[:], var[:], AF.Sqrt)
                        rstd = nod.tile([P, 1], dt.float32, tag="rstd")
                        nc.vector.reciprocal(out=rstd[:], in_=std[:])
                        of = nod.tile([P, H], dt.float32, tag="of")
                        nc.vector.tensor_scalar_mul(of[:], xc[:], rstd[:])
                        if not meta["ln_id"]:
                            nc.vector.tensor_mul(out=of[:], in0=of[:],
                                                 in1=lng[:])
                            nc.vector.tensor_add(out=of[:], in0=of[:],
                                                 in1=lnb[:])
                        # int8 quantization with per-row scale
                        rmax = nod.tile([P, 1], dt.float32, tag="rmax")
                        nc.vector.reduce_max(out=rmax[:], in_=of[:],
                                             axis=mybir.AxisListType.X,
                                             apply_absolute_value=True)
                        nc.vector.tensor_scalar(
                            out=rmax[:], in0=rmax[:], scalar1=1e-4,
                            scalar2=0, op0=ALU.max, op1=ALU.bypass)
                        inv = nod.tile([P, 1], dt.float32, tag="inv")
                        nc.vector.reciprocal(out=inv[:], in_=rmax[:])
                        qf = nod.tile([P, H], dt.float32, tag="qf")
                        nc.vector.tensor_scalar(
                            out=qf[:], in0=of[:], scalar1=inv[:],
                            scalar2=127.0, op0=ALU.mult, op1=ALU.mult)
                        q8 = nod.tile([P, H], dt.int8, tag="q8")
                        nc.vector.tensor_copy(out=q8[:], in_=qf[:])
                        s16 = nod.tile([P, 1], dt.float16, tag="s16")
                        nc.vector.tensor_scalar_mul(s16[:], rmax[:],
                                                    1.0 / 127.0)
                        nc.sync.dma_start(outq_d[w * P:(w + 1) * P, :],
                                          q8[:])
                        nc.sync.dma_start(outs_d[w * P:(w + 1) * P, :],
                                          s16[:])
    nc.finalize()
    return nc


def kernel(**inputs):
    from concourse.bass_utils import run_bass_kernel_spmd

    ins_per_core, meta, N = _prep_host(**inputs)
    key = (meta["NT"], meta["T"], meta["has_eb2"], meta["has_nb2"],
           meta["ln_id"])
    if key not in _CACHE:
        _CACHE[key] = _build_nc(meta)
    nc = _CACHE[key]
    res = run_bass_kernel_spmd(nc, ins_per_core, list(range(NCORES)))
    global _LAST_EXEC_NS
    _LAST_EXEC_NS = getattr(res, "exec_time_ns", None)
    outs = []
    for c in range(NCORES):
        q = np.asarray(res.results[c]["outq"]).astype(np.float32)
        s = np.asarray(res.results[c]["outs"]).astype(np.float32)
        outs.append(q * s)
    full = np.concatenate(outs, axis=0)[:N]
    return full.astype(np.float32)


# revision 16
# speedup vs baseline: 1.4835x; 1.4835x over previous
"""Trainium2 Bass kernel for ColorFlowLayer GNN message passing.

Design (8 NeuronCores, SPMD; wall-clock over the axon tunnel is the
bottleneck at ~42 MB/s, so the kernel minimizes host<->device bytes and
the number of transferred arrays):
  - Ship per-core ONLY two packed blobs: fp16 (h shard, window one-hot
    keys, folded weights) and uint16 (edge indices; rel packed into the
    high 3 bits of dst).  ~2.5 MB/core in, 1.6 MB/core out.
  - On device: AllGather h shards (DRAM bounce -> Shared), build fp16
    feature-major tables in SBUF:
      A.T [128, 50176]  = (h @ eW1[0:128]).T    (+ role/color combos)
      B'.T [128, 6272]  = (h_loc @ eW1[128:256]).T (+ role/color combos)
      znh.T [128, 6272] = (h_loc @ nW1[0:128]).T (+ role/color + nb1)
    Role/color/rel embedding contributions enter via 18/8-column combo
    tables gathered by host-precomputed uint16 codes.
  - Edge phase: edges sorted by dst, padded into 128-edge tiles that
    never span a 128-node window; per 1024-edge block, gpsimd
    indirect_copy gathers per-edge columns of A.T (7 zero-padded
    chunks <= 8066 cols, summed), B'.T, Rtab.T; z = sum -> silu ->
    y1 @ eW2 -> silu -> one-hot segment-sum matmul into PSUM.
  - Node phase per 128-node window: zn = nW1_agg.T @ agg + znh ->
    silu -> @ nW2 -> + h -> LayerNorm (f32) -> fp16 out.
"""

import numpy as np

H = 128
P = 128
NCORES = 8
NS = 6272            # padded nodes per core = 49 windows * 128
NWL = NS // P        # 49 local windows
NWG = NWL * NCORES   # 392 global windows
NG = NS * NCORES     # 50176 padded global nodes
BLK = 8              # edge tiles per block (1024 edges)
CH = 8064            # A-table chunk data columns (63 windows)
NCHUNK = 7           # 6*8064 + 1792 = 50176
LCH = NG - 6 * CH    # 1792, last chunk data cols
LN_EPS = 1e-5

_CACHE = {}
_LAST_EXEC_NS = None


def _offsets(NT):
    """Row offsets (rows of 128 elements) into the two packed blobs."""
    oF = {}
    r = 0
    for name, rows in [("h16", NS), ("W1_hs", H),
                       ("W1_hd", H), ("eW2", H), ("nW1_h", H),
                       ("nW1_agg", H), ("nW2", H), ("RtabT", 8),
                       ("ACT", 18), ("BCT", 18), ("NTT", 18),
                       ("eb2row", 1), ("nb2row", 1), ("lng", 1), ("lnb", 1)]:
        oF[name] = (r, rows)
        r += rows
    oU = {}
    r = 0
    for name, rows in [("srcv", NT), ("dstrv", NT), ("rc_all", NWG),
                       ("rc_loc", NWL)]:
        oU[name] = (r, rows)
        r += rows
    return oF, oU


def _prep_host(h, edge_index, edge_relation, node_color_rep, node_role,
               rel_emb, role_emb, color_emb,
               eW1, eb1, eW2, eb2, nW1, nb1, nW2, nb2, ln_g, ln_b):
    f32, f16, u16 = np.float32, np.float16, np.uint16
    h = np.asarray(h, f32)
    src = np.asarray(edge_index[0]).astype(np.int32)
    dst = np.asarray(edge_index[1]).astype(np.int32)
    rel = np.asarray(edge_relation).astype(np.int32)
    role = np.asarray(node_role).astype(np.int32)
    col = np.asarray(node_color_rep).astype(np.int32)
    N = h.shape[0]
    E = src.shape[0]

    # ---- folded weights (tiny) ----
    eW1 = np.asarray(eW1, f32)
    nW1 = np.asarray(nW1, f32)
    Rtab = np.asarray(rel_emb, f32) @ eW1[256:272] + np.asarray(eb1, f32)
    RA = np.asarray(role_emb, f32) @ eW1[272:280]
    RB = np.asarray(role_emb, f32) @ eW1[280:288]
    CA = np.asarray(color_emb, f32) @ eW1[288:296]
    CB = np.asarray(color_emb, f32) @ eW1[296:304]
    NR = np.asarray(role_emb, f32) @ nW1[256:264] + np.asarray(nb1, f32)
    NC = np.asarray(color_emb, f32) @ nW1[264:272]
    AC18 = (RA[:, None, :] + CA[None, :, :]).reshape(18, H)
    BC18 = (RB[:, None, :] + CB[None, :, :]).reshape(18, H)
    NT18 = (NR[:, None, :] + NC[None, :, :]).reshape(18, H)

    eb2 = np.asarray(eb2, f32)
    nb2 = np.asarray(nb2, f32)
    has_eb2 = bool(np.any(eb2 != 0))
    has_nb2 = bool(np.any(nb2 != 0))
    ln_g = np.asarray(ln_g, f32)
    ln_b = np.asarray(ln_b, f32)
    ln_id = bool(np.all(ln_g == 1) and np.all(ln_b == 0))

    # ---- edge sharding: group by 128-node window (core = dst // NS) ----
    gw0 = dst >> 7                        # global window id 0..391
    o = np.argsort(gw0, kind="stable")    # 2-pass radix on small keys
    src_s, dst_s, rel_s = src[o], dst[o], rel[o]
    gw = gw0[o]
    core_s = gw // NWL
    wcnt = np.bincount(gw0, minlength=NWG)
    cnts = wcnt.reshape(NCORES, NWL)      # [core, local window]
    T = np.maximum(1, np.ceil(cnts.max(axis=0) / P).astype(np.int64))
    NT = int(T.sum())
    NT += (-NT) % BLK
    T[NWL - 1] += NT - int(T.sum())
    offs = np.concatenate([[0], np.cumsum(T)]).astype(np.int64)
    NB = NT // BLK

    starts = np.concatenate([[0], np.cumsum(wcnt)]).astype(np.int64)
    rank = np.arange(E, dtype=np.int64) - starts[gw]
    slot = offs[gw % NWL] * P + rank      # slot within the core's edge space

    srcv = np.zeros((NCORES, NT * P), u16)
    dstrv = np.full((NCORES, NT * P), 8191, u16)   # pad marker: dst=8191
    srcv[core_s, slot] = src_s.astype(u16)
    dstrv[core_s, slot] = (rel_s * 8192 + dst_s - core_s * NS).astype(u16)

    def tilecm(a):  # [NT*P] slot-major -> [P, NT] (slot t*128+p at [p, t])
        return np.ascontiguousarray(a.reshape(NT, P).T)

    # ---- per-node role-color codes ----
    rc = np.zeros(NG, u16)
    rc[:N] = (role * 3 + col).astype(u16)
    rc_all = np.ascontiguousarray(rc.reshape(NWG, P).T)  # [P, 392]

    h_pad = np.zeros((NG, H), f16)
    h_pad[:N] = h.astype(f16)

    oF, oU = _offsets(NT)
    rowsF = sum(n for _, n in oF.values())
    rowsU = sum(n for _, n in oU.values())

    fixedF = {
        "W1_hs": eW1[0:128].astype(f16),
        "W1_hd": eW1[128:256].astype(f16),
        "eW2": np.asarray(eW2, f32).astype(f16),
        "nW1_h": nW1[0:128].astype(f16),
        "nW1_agg": nW1[128:256].astype(f16),
        "nW2": np.asarray(nW2, f32).astype(f16),
        "RtabT": np.ascontiguousarray(Rtab.T).astype(f16),
        "ACT": np.ascontiguousarray(AC18.T).astype(f16),
        "BCT": np.ascontiguousarray(BC18.T).astype(f16),
        "NTT": np.ascontiguousarray(NT18.T).astype(f16),
        "eb2row": eb2.reshape(1, H).astype(f16),
        "nb2row": nb2.reshape(1, H).astype(f16),
        "lng": ln_g.reshape(1, H).astype(f16),
        "lnb": ln_b.reshape(1, H).astype(f16),
    }

    ins_per_core = []
    for c in range(NCORES):
        blobF = np.empty((rowsF, P), f16)
        blobU = np.empty((rowsU, P), u16)

        def putF(name, arr):
            r0, nr = oF[name]
            blobF[r0:r0 + nr] = np.asarray(arr).reshape(nr, P)

        def putU(name, arr):
            r0, nr = oU[name]
            blobU[r0:r0 + nr] = np.asarray(arr).reshape(nr, P)

        putF("h16", h_pad[c * NS:(c + 1) * NS])
        for kk, vv in fixedF.items():
            putF(kk, vv)
        putU("srcv", tilecm(srcv[c]))
        putU("dstrv", tilecm(dstrv[c]))
        putU("rc_all", rc_all)
        putU("rc_loc", rc_all[:, c * NWL:(c + 1) * NWL])
        ins_per_core.append(dict(blobF=blobF, blobU=blobU))
    meta = dict(NT=NT, T=tuple(int(t) for t in T),
                has_eb2=has_eb2, has_nb2=has_nb2, ln_id=ln_id)
    return ins_per_core, meta, N


def _build_nc(meta):
    import concourse.bass as bass
    import concourse.bacc as bacc
    import concourse.mybir as mybir
    import concourse.tile as tile
    from concourse.masks import make_identity
    from contextlib import ExitStack

    NT = meta["NT"]
    T = meta["T"]
    NB = NT // BLK
    AF = mybir.ActivationFunctionType
    ALU = mybir.AluOpType
    dt = mybir.dt
    nc = bacc.Bacc()

    oF, oU = _offsets(NT)
    rowsF = sum(n for _, n in oF.values())
    rowsU = sum(n for _, n in oU.values())
    blobF_d = nc.dram_tensor("blobF", [rowsF, P], dt.float16,
                             kind="ExternalInput")
    blobU_d = nc.dram_tensor("blobU", [rowsU, P], dt.uint16,
                             kind="ExternalInput")

    def fsl(name):
        r0, nr = oF[name]
        return blobF_d[r0:r0 + nr, :]

    def usl(name):
        r0, nr = oU[name]
        return blobU_d[r0:r0 + nr, :]

    outq_d = nc.dram_tensor("outq", [NS, H], dt.int8, kind="ExternalOutput")
    outs_d = nc.dram_tensor("outs", [NS, 1], dt.float16,
                            kind="ExternalOutput")

    ag_in = nc.dram_tensor("ag_in", [NS, H], dt.float16)
    h_all = nc.dram_tensor("h_all", [NG, H], dt.float16,
                           addr_space="Shared")

    ACH = 6 * (CH + 2) + (LCH + 2)   # flat A-table cols incl zero pads

    with tile.TileContext(nc) as tc, ExitStack() as ctx:
        cst = ctx.enter_context(tc.tile_pool(name="cst", bufs=1))
        big = ctx.enter_context(tc.tile_pool(name="big", bufs=1))

        # ---- start the collective as early as possible ----
        nc.sync.dma_start(ag_in[:], fsl("h16"))
        tc.strict_bb_all_engine_barrier()
        nc.gpsimd.collective_compute(
            "AllGather", mybir.AluOpType.bypass,
            replica_groups=[list(range(NCORES))],
            ins=[ag_in[:]], outs=[h_all[:]])

        # ---- constants ----
        ident = cst.tile([P, P], dt.float16)
        make_identity(nc, ident[:])
        W1_hs = cst.tile([H, H], dt.float16)
        W1_hd = cst.tile([H, H], dt.float16)
        eW2 = cst.tile([H, H], dt.float16)
        nW1_h = cst.tile([H, H], dt.float16)
        nW1_agg = cst.tile([H, H], dt.float16)
        nW2 = cst.tile([H, H], dt.float16)
        RtabT = cst.tile([P, 8], dt.float16)
        ACT = cst.tile([P, 18], dt.float16)
        BCT = cst.tile([P, 18], dt.float16)
        NTT = cst.tile([P, 18], dt.float16)
        eb2r = cst.tile([1, H], dt.float16)
        nb2r = cst.tile([1, H], dt.float16)
        lngr = cst.tile([1, H], dt.float16)
        lnbr = cst.tile([1, H], dt.float16)
        ones1h = cst.tile([1, P], dt.float16)
        for nm, dstt in [("W1_hs", W1_hs), ("W1_hd", W1_hd), ("eW2", eW2),
                         ("nW1_h", nW1_h), ("nW1_agg", nW1_agg),
                         ("nW2", nW2), ("RtabT", RtabT), ("ACT", ACT),
                         ("BCT", BCT), ("NTT", NTT), ("eb2row", eb2r),
                         ("nb2row", nb2r), ("lng", lngr), ("lnb", lnbr)]:
            nc.sync.dma_start(dstt[:], fsl(nm))
        nc.vector.memset(ones1h[:], 1.0)

        iota16 = cst.tile([P, BLK, P], dt.float16)

        # LN gamma/beta broadcast to [P, H] f32 via ones-matmul
        lng = cst.tile([P, H], dt.float32)
        lnb = cst.tile([P, H], dt.float32)
        with tc.tile_pool(name="lnp", bufs=2, space="PSUM") as lnp:
            pg = lnp.tile([P, H], dt.float32, tag="g")
            nc.tensor.matmul(out=pg[:], lhsT=ones1h[:], rhs=lngr[:],
                             start=True, stop=True)
            nc.vector.tensor_copy(out=lng[:], in_=pg[:])
            pb = lnp.tile([P, H], dt.float32, tag="b")
            nc.tensor.matmul(out=pb[:], lhsT=ones1h[:], rhs=lnbr[:],
                             start=True, stop=True)
            nc.vector.tensor_copy(out=lnb[:], in_=pb[:])

        # ---- persistent tables ----
        tblA = big.tile([P, ACH], dt.float16)
        nc.vector.memset(tblA[:], 0.0)
        tblB = big.tile([P, NS], dt.float16)
        znh = big.tile([P, NS], dt.float16)
        h_raw = big.tile([P, NWL, H], dt.float16)
        dstwv_w = big.tile([P, NB, BLK], dt.float16)
        srcW = big.tile([P, NB, 64], dt.uint16)
        dstW = big.tile([P, NB, 64], dt.uint16)
        relW = big.tile([P, NB, 64], dt.uint16)
        WrcA = big.tile([P, NWG, 8], dt.uint16)
        WrcL = big.tile([P, NWL, 8], dt.uint16)
        with tc.tile_pool(name="stg", bufs=1) as stg:
            iota_g = stg.tile([P, BLK, P], dt.float16)
            nc.gpsimd.iota(iota_g[:], pattern=[[0, BLK], [1, P]], base=0,
                           channel_multiplier=0,
                           allow_small_or_imprecise_dtypes=True)
            # DVE-owned copy: the 3D-broadcast is_equal only has room for
            # one sync wait, so both inputs must come from DVE producers.
            nc.vector.tensor_copy(out=iota16[:], in_=iota_g[:])
            srcv = stg.tile([P, NB, BLK], dt.uint16)
            dstvt = stg.tile([P, NB, BLK], dt.uint16)
            rc_all = stg.tile([P, NWG], dt.uint16)
            rc_loc = stg.tile([P, NWL], dt.uint16)
            nc.sync.dma_start(srcv[:], usl("srcv"))
            nc.sync.dma_start(dstvt[:], usl("dstrv"))
            nc.sync.dma_start(rc_all[:], usl("rc_all"))
            nc.sync.dma_start(rc_loc[:], usl("rc_loc"))
            # W[q, blk, tl*8+cp] = v[cp*16+q, blk, tl]
            for tl in range(BLK):
                for cp in range(8):
                    c = tl * 8 + cp
                    sl = slice(cp * 16, cp * 16 + 16)
                    nc.sync.dma_start(srcW[0:16, :, c:c + 1],
                                      srcv[sl, :, tl:tl + 1])
                    nc.sync.dma_start(dstW[0:16, :, c:c + 1],
                                      dstvt[sl, :, tl:tl + 1])
            for g in range(1, 8):
                gs = slice(16 * g, 16 * (g + 1))
                nc.sync.dma_start(srcW[gs, :, :], srcW[0:16, :, :])
                nc.sync.dma_start(dstW[gs, :, :], dstW[0:16, :, :])
            # unpack rel (high 3 bits) out of dstW; clamp pads to 6271
            nc.vector.tensor_scalar(out=relW[:], in0=dstW[:], scalar1=13,
                                    scalar2=0,
                                    op0=ALU.logical_shift_right,
                                    op1=ALU.bypass)
            nc.vector.tensor_scalar(out=dstW[:], in0=dstW[:], scalar1=8191,
                                    scalar2=0,
                                    op0=ALU.bitwise_and, op1=ALU.bypass)
            nc.vector.tensor_scalar(out=dstW[:], in0=dstW[:],
                                    scalar1=NS - 1, scalar2=0,
                                    op0=ALU.min, op1=ALU.bypass)
            # derive window one-hot keys from compact dstrv:
            # m = dst&8191; c = (m < NS); dstwv = (m&127 + 1)*c - 1
            dm = stg.tile([P, NB, BLK], dt.uint16)
            nc.vector.tensor_scalar(out=dm[:], in0=dstvt[:], scalar1=8191,
                                    scalar2=0, op0=ALU.bitwise_and,
                                    op1=ALU.bypass)
            dc = stg.tile([P, NB, BLK], dt.uint16)
            nc.vector.tensor_scalar(out=dc[:], in0=dm[:], scalar1=NS,
                                    scalar2=0, op0=ALU.is_lt,
                                    op1=ALU.bypass)
            dw1 = stg.tile([P, NB, BLK], dt.uint16)
            nc.vector.tensor_scalar(out=dw1[:], in0=dm[:], scalar1=127,
                                    scalar2=0, op0=ALU.bitwise_and,
                                    op1=ALU.bypass)
            nc.vector.tensor_scalar(out=dw1[:], in0=dw1[:], scalar1=1,
                                    scalar2=0, op0=ALU.add,
                                    op1=ALU.bypass)
            dwf = stg.tile([P, NB, BLK], dt.float16)
            nc.vector.tensor_tensor(out=dwf[:], in0=dw1[:], in1=dc[:],
                                    op=ALU.mult)
            nc.vector.tensor_scalar(out=dstwv_w[:], in0=dwf[:],
                                    scalar1=1.0, scalar2=0.0,
                                    op0=ALU.subtract, op1=ALU.bypass)
            # window-granular wraps for rc: Wrc[q, w, cp] = rc[cp*16+q, w]
            for cp in range(8):
                sl = slice(cp * 16, cp * 16 + 16)
                nc.sync.dma_start(WrcA[0:16, :, cp:cp + 1],
                                  rc_all[sl, :].unsqueeze(2))
                nc.sync.dma_start(WrcL[0:16, :, cp:cp + 1],
                                  rc_loc[sl, :].unsqueeze(2))
            for g in range(1, 8):
                gs = slice(16 * g, 16 * (g + 1))
                nc.sync.dma_start(WrcA[gs, :, :], WrcA[0:16, :, :])
                nc.sync.dma_start(WrcL[gs, :, :], WrcL[0:16, :, :])

        # ---- local phase: h_raw, tblB, znh from h16 ----
        h16r0 = oF["h16"][0]
        with tc.tile_pool(name="tpp", bufs=2, space="PSUM") as tpp, \
             tc.tile_pool(name="tbp", bufs=2, space="PSUM") as tbp, \
             tc.tile_pool(name="lsb", bufs=3) as lsb:
            for w in range(NWL):
                nc.sync.dma_start(
                    h_raw[:, w, :],
                    blobF_d[h16r0 + w * P:h16r0 + (w + 1) * P, :])
                pt = tpp.tile([P, P], dt.float16, tag="tr")
                nc.tensor.transpose(out=pt[:], in_=h_raw[:, w, :],
                                    identity=ident[:])
                hT = lsb.tile([P, P], dt.float16, tag="hT")
                nc.vector.tensor_copy(out=hT[:], in_=pt[:])
                pb = tbp.tile([P, P], dt.float32, tag="pb")
                nc.tensor.matmul(out=pb[:], lhsT=W1_hd[:], rhs=hT[:],
                                 start=True, stop=True)
                gb = lsb.tile([P, P], dt.float16, tag="gb")
                nc.gpsimd.indirect_copy(out=gb[:], data=BCT[:],
                                        idxs=WrcL[:, w, :],
                                        i_know_ap_gather_is_preferred=True)
                nc.vector.tensor_add(out=tblB[:, w * P:(w + 1) * P],
                                     in0=pb[:], in1=gb[:])
                pz = tbp.tile([P, P], dt.float32, tag="pz")
                nc.tensor.matmul(out=pz[:], lhsT=nW1_h[:], rhs=hT[:],
                                 start=True, stop=True)
                gz = lsb.tile([P, P], dt.float16, tag="gz")
                nc.gpsimd.indirect_copy(out=gz[:], data=NTT[:],
                                        idxs=WrcL[:, w, :],
                                        i_know_ap_gather_is_preferred=True)
                nc.vector.tensor_add(out=znh[:, w * P:(w + 1) * P],
                                     in0=pz[:], in1=gz[:])

        tc.strict_bb_all_engine_barrier()

        # ---- global phase: tblA from h_all ----
        with tc.tile_pool(name="gpp", bufs=2, space="PSUM") as gpp, \
             tc.tile_pool(name="gap", bufs=2, space="PSUM") as gap, \
             tc.tile_pool(name="gsb", bufs=3) as gsb:
            for w in range(NWG):
                k = w // 63
                off = k * (CH + 2) + (w % 63) * P + 1
                ht = gsb.tile([P, P], dt.float16, tag="ld")
                nc.sync.dma_start(ht[:], h_all[w * P:(w + 1) * P, :])
                pt = gpp.tile([P, P], dt.float16, tag="tr")
                nc.tensor.transpose(out=pt[:], in_=ht[:], identity=ident[:])
                hT = gsb.tile([P, P], dt.float16, tag="hT")
                nc.vector.tensor_copy(out=hT[:], in_=pt[:])
                pa = gap.tile([P, P], dt.float32, tag="pa")
                nc.tensor.matmul(out=pa[:], lhsT=W1_hs[:], rhs=hT[:],
                                 start=True, stop=True)
                ga = gsb.tile([P, P], dt.float16, tag="ga")
                nc.gpsimd.indirect_copy(out=ga[:], data=ACT[:],
                                        idxs=WrcA[:, w, :],
                                        i_know_ap_gather_is_preferred=True)
                nc.vector.tensor_add(out=tblA[:, off:off + P],
                                     in0=pa[:], in1=ga[:])

        # ---- edge + node phases ----
        w_first = {}
        w_last = {}
        t2w = []
        for w in range(NWL):
            for _ in range(T[w]):
                t2w.append(w)
        for t, w in enumerate(t2w):
            w_first.setdefault(w, t)
            w_last[w] = t

        with tc.tile_pool(name="wkp", bufs=3) as wkp, \
             tc.tile_pool(name="zp", bufs=2) as zp, \
             tc.tile_pool(name="gp", bufs=1) as gp, \
             tc.tile_pool(name="y1p", bufs=2) as y1p, \
             tc.tile_pool(name="ohp", bufs=2) as ohp, \
             tc.tile_pool(name="msp", bufs=2) as msp, \
             tc.tile_pool(name="nod", bufs=1) as nod, \
             tc.tile_pool(name="mps", bufs=2, space="PSUM") as mps, \
             tc.tile_pool(name="aps", bufs=1, space="PSUM") as aps, \
             tc.tile_pool(name="nps", bufs=2, space="PSUM") as nps:

            agg_ps = None
            for b in range(NB):
                t0 = b * BLK
                z = zp.tile([P, BLK * P], dt.float16, tag="z")
                gt = gp.tile([P, BLK * P], dt.float16, tag="gt")
                for k in range(NCHUNK):
                    wk = wkp.tile([P, 64], dt.uint16, tag="wk")
                    lim = (CH + 1) if k < 6 else (LCH + 1)
                    if k == 0:
                        nc.vector.tensor_scalar(
                            out=wk[:], in0=srcW[:, b, :], scalar1=1,
                            scalar2=lim, op0=ALU.add, op1=ALU.min)
                    else:
                        nc.vector.tensor_scalar(
                            out=wk[:], in0=srcW[:, b, :],
                            scalar1=k * CH - 1, scalar2=lim,
                            op0=ALU.subtract, op1=ALU.min)
                    koff = k * (CH + 2)
                    klen = (CH + 2) if k < 6 else (LCH + 2)
                    dslice = tblA[:, koff:koff + klen]
                    if k == 0:
                        nc.gpsimd.indirect_copy(
                            out=z[:], data=dslice, idxs=wk[:],
                            i_know_ap_gather_is_preferred=True)
                    else:
                        nc.gpsimd.indirect_copy(
                            out=gt[:], data=dslice, idxs=wk[:],
                            i_know_ap_gather_is_preferred=True)
                        nc.vector.tensor_add(out=z[:], in0=z[:], in1=gt[:])
                gb = gp.tile([P, BLK * P], dt.float16, tag="gb")
                nc.gpsimd.indirect_copy(
                    out=gb[:], data=tblB[:], idxs=dstW[:, b, :],
                    i_know_ap_gather_is_preferred=True)
                nc.vector.tensor_add(out=z[:], in0=z[:], in1=gb[:])
                gr = gp.tile([P, BLK * P], dt.float16, tag="gr")
                nc.gpsimd.indirect_copy(
                    out=gr[:], data=RtabT[:], idxs=relW[:, b, :],
                    i_know_ap_gather_is_preferred=True)
                nc.vector.tensor_add(out=z[:], in0=z[:], in1=gr[:])

                y1 = y1p.tile([P, BLK * P], dt.float16, tag="y1")
                nc.scalar.activation(y1[:], z[:], AF.Silu)

                oh = ohp.tile([P, BLK, P], dt.float16, tag="oh")
                nc.vector.tensor_tensor(
                    out=oh[:],
                    in0=dstwv_w[:, b, :].unsqueeze(2).to_broadcast(
                        [P, BLK, P]),
                    in1=iota16[:],
                    op=ALU.is_equal)

                for half in range(2):
                    mp = mps.tile([P, 4 * P], dt.float32, tag="m")
                    for s4 in range(4):
                        s = half * 4 + s4
                        nc.tensor.matmul(out=mp[:, s4 * P:(s4 + 1) * P],
                                         lhsT=y1[:, s * P:(s + 1) * P],
                                         rhs=eW2[:],
                                         start=True, stop=not meta["has_eb2"])
                        if meta["has_eb2"]:
                            nc.tensor.matmul(out=mp[:, s4 * P:(s4 + 1) * P],
                                             lhsT=ones1h[:], rhs=eb2r[:],
                                             start=False, stop=True)
                    ms = msp.tile([P, 4 * P], dt.float16, tag="ms")
                    nc.scalar.activation(ms[:], mp[:], AF.Silu)
                    for s4 in range(4):
                        s = half * 4 + s4
                        t = t0 + s
                        w = t2w[t]
                        if t == w_first[w]:
                            agg_ps = aps.tile([P, P], dt.float32, tag="agg")
                        nc.tensor.matmul(out=agg_ps[:],
                                         lhsT=ms[:, s4 * P:(s4 + 1) * P],
                                         rhs=oh[:, s, :],
                                         start=(t == w_first[w]),
                                         stop=(t == w_last[w]))
                        if t != w_last[w]:
                            continue
                        # ---------- node phase for window w ----------
                        aggT = nod.tile([P, P], dt.float16, tag="aggT")
                        nc.vector.tensor_copy(out=aggT[:], in_=agg_ps[:])
                        zn = nps.tile([P, P], dt.float32, tag="zn")
                        nc.tensor.matmul(out=zn[:], lhsT=nW1_agg[:],
                                         rhs=aggT[:], start=True, stop=True)
                        zs = nod.tile([P, P], dt.float16, tag="zs")
                        nc.vector.tensor_add(
                            out=zs[:], in0=znh[:, w * P:(w + 1) * P],
                            in1=zn[:])
                        y1n = nod.tile([P, P], dt.float16, tag="y1n")
                        nc.scalar.activation(y1n[:], zs[:], AF.Silu)
                        up = nps.tile([P, P], dt.float32, tag="up")
                        nc.tensor.matmul(out=up[:], lhsT=y1n[:], rhs=nW2[:],
                                         start=True,
                                         stop=not meta["has_nb2"])
                        if meta["has_nb2"]:
                            nc.tensor.matmul(out=up[:], lhsT=ones1h[:],
                                             rhs=nb2r[:], start=False,
                                             stop=True)
                        x = nod.tile([P, H], dt.float32, tag="x")
                        nc.vector.tensor_add(out=x[:], in0=up[:],
                                             in1=h_raw[:, w, :])
                        mu = nod.tile([P, 1], dt.float32, tag="mu")
                        nc.vector.reduce_sum(out=mu[:], in_=x[:],
                                             axis=mybir.AxisListType.X)
                        nc.vector.tensor_scalar_mul(mu[:], mu[:], -1.0 / H)
                        xc = nod.tile([P, H], dt.float32, tag="xc")
                        nc.vector.tensor_scalar_add(xc[:], x[:], mu[:])
                        sq = nod.tile([P, H], dt.float32, tag="sq")
                        nc.vector.tensor_mul(out=sq[:], in0=xc[:], in1=xc[:])
                        var = nod.tile([P, 1], dt.float32, tag="var")
                        nc.vector.reduce_sum(out=var[:], in_=sq[:],
                                             axis=mybir.AxisListType.X)
                        nc.vector.tensor_scalar(
                            out=var[:], in0=var[:],
                            scalar1=1.0 / H, scalar2=LN_EPS,
                            op0=ALU.mult, op1=ALU.add)
                        std = nod.tile([P, 1], dt.float32, tag="std")
                        nc.scalar.activation(std[:], var[:], AF.Sqrt)
                        rstd = nod.tile([P, 1], dt.float32, tag="rstd")
                        nc.vector.reciprocal(out=rstd[:], in_=std[:])
                        of = nod.tile([P, H], dt.float32, tag="of")
                        nc.vector.tensor_scalar_mul(of[:], xc[:], rstd[:])
                        if not meta["ln_id"]:
                            nc.vector.tensor_mul(out=of[:], in0=of[:],
                                                 in1=lng[:])
                            nc.vector.tensor_add(out=of[:], in0=of[:],
                                                 in1=lnb[:])
                        # int8 quantization with per-row scale
                        rmax = nod.tile([P, 1], dt.float32, tag="rmax")
                        nc.vector.reduce_max(out=rmax[:], in_=of[:],
                                             axis=mybir.AxisListType.X,
                                             apply_absolute_value=True)
                        nc.vector.tensor_scalar(
                            out=rmax[:], in0=rmax[:], scalar1=1e-4,
                            scalar2=0, op0=ALU.max, op1=ALU.bypass)
                        inv = nod.tile([P, 1], dt.float32, tag="inv")
                        nc.vector.reciprocal(out=inv[:], in_=rmax[:])
                        qf = nod.tile([P, H], dt.float32, tag="qf")
                        nc.vector.tensor_scalar(
                            out=qf[:], in0=of[:], scalar1=inv[:],
                            scalar2=127.0, op0=ALU.mult, op1=ALU.mult)
                        q8 = nod.tile([P, H], dt.int8, tag="q8")
                        nc.vector.tensor_copy(out=q8[:], in_=qf[:])
                        s16 = nod.tile([P, 1], dt.float16, tag="s16")
                        nc.vector.tensor_scalar_mul(s16[:], rmax[:],
                                                    1.0 / 127.0)
                        nc.sync.dma_start(outq_d[w * P:(w + 1) * P, :],
                                          q8[:])
                        nc.sync.dma_start(outs_d[w * P:(w + 1) * P, :],
                                          s16[:])
    nc.finalize()
    return nc


def kernel(**inputs):
    from concourse.bass_utils import run_bass_kernel_spmd

    ins_per_core, meta, N = _prep_host(**inputs)
    key = (meta["NT"], meta["T"], meta["has_eb2"], meta["has_nb2"],
           meta["ln_id"])
    if key not in _CACHE:
        _CACHE[key] = _build_nc(meta)
    nc = _CACHE[key]
    res = run_bass_kernel_spmd(nc, ins_per_core, list(range(NCORES)))
    global _LAST_EXEC_NS
    _LAST_EXEC_NS = getattr(res, "exec_time_ns", None)
    outs = []
    for c in range(NCORES):
        q = np.asarray(res.results[c]["outq"]).astype(np.float32)
        s = np.asarray(res.results[c]["outs"]).astype(np.float32)
        outs.append(q * s)
    full = np.concatenate(outs, axis=0)[:N]
    return full.astype(np.float32)
